# revision 29
# baseline (speedup 1.0000x reference)
"""Trainium2 Bass kernel for nn_Attention_51823075393746.

Self-attention block (SAGAN-style) over x:[16,128,64,64]:
  theta = w_theta @ x            [B, 16, 4096]
  phi   = pool2x2(w_phi @ x)     [B, 16, 1024]
  g     = pool2x2(w_g @ x)       [B, 64, 1024]
  beta  = softmax(theta^T phi)   [B, 4096, 1024]
  out   = gamma * (w_o @ (g @ beta^T)) + x

Sharding: data-parallel over batch, 2 samples per core on 8 cores.

Redesign driven by HW traces (baseline 206us -> ~150us):
  - ACT exp of the 4M-element attention matrix is the hard floor
    (64x [128,2,512] ACTIVATEs ~1.1us each), so the pipeline keeps ACT
    fed and every other engine's work below the PE's.
  - scores matmuls (K=16) are 2-way row-tiled via tile_position: phi
    k-even tiles live at partitions 0:16, k-odd at 32:48, theta is
    replicated to both groups by duplicating w_theta rows inside the
    projection weight; the two matmuls of a pair run CONCURRENTLY on
    disjoint 32-row PE groups (measured: starts 4ns apart).
  - attention runs in j-blocks of 2 chunks; phi/ga k-slices serve the
    two chunks back-to-back, and the o-matmuls for pair q-1 are emitted
    after the scores of pair q, so the PE always has runnable work while
    ACT drains the exp backlog (no head-of-queue blocking).  A dense PE
    stream also keeps the HAM clock gate at 2.4GHz.
  - PSUM (8 banks): scores 2x[128,2,512] + po(j0,j1) 2x[128,512] +
    proj/transpose/wo shared 2x[128,512].  theta/ga PSUM->SBUF copies
    run on ACT (idle during projection); pools on DVE.  NOTE: engine
    PSUM reads must start at a 32-aligned partition (compile-enforced),
    hence the padded projection-weight row layout.
  - softmax: compile-time shift exp(s-6); denominator = ones column in
    ga -> row 64 of po, kept fp32 through a per-block reciprocal +
    DRAM-roundtrip DMA broadcast (GPSIMD partition_broadcast and a
    reciprocal reading from partition base 64 give WRONG RESULTS on HW
    despite passing CoreSim); normalization commutes through w_o.
  - residual from the fp16 x copy on-chip: out = w_o @ o * (1/den) +
    x16, one plain store, no fp32 x load, no DMA read-modify-write.
"""

import sys

for _p in ("/opt/trn_rl_repo",):
    if _p not in sys.path:
        sys.path.insert(0, _p)

import numpy as np

import concourse.bass as bass
import concourse.bacc as bacc
import concourse.mybir as mybir
import concourse.tile as tile

F32 = mybir.dt.float32
F16 = mybir.dt.float16
AF = mybir.ActivationFunctionType
ALU = mybir.AluOpType

B, C, H, W = 16, 128, 64, 64
N = H * W          # 4096 spatial positions
M = N // 4         # 1024 pooled positions
CT = 16            # theta/phi channels (C//8)
CG = 64            # g channels (C//2)
NCORES = 8
NS = B // NCORES   # samples per core
NC = 512           # spatial chunk (free dim of matmuls)
NJ = N // NC       # 8 chunks
KM = M // 128      # 8 m-tiles of pooled positions
PR = NC // 4       # pooled positions produced per chunk (128)
K_SHIFT = 6.0      # constant softmax shift: exp(score - K) keeps fp16 happy


def build_nc(ns: int = NS) -> bass.Bass:
    nc = bacc.Bacc()
    x16d = nc.dram_tensor("x16", [ns, C, N], F16, kind="ExternalInput")
    wtd = nc.dram_tensor("wt16", [C, C], F16, kind="ExternalInput")
    wt2d = nc.dram_tensor("wt2", [C, C], F16, kind="ExternalInput")
    wod = nc.dram_tensor("wo16", [CG, C], F16, kind="ExternalInput")
    onec = nc.dram_tensor("onec", [C, KM, CG], F16, kind="ExternalInput")
    identd = nc.dram_tensor("ident", [CG, CG], F16, kind="ExternalInput")
    out = nc.dram_tensor("out", [ns, C, N], F32, kind="ExternalOutput")

    with tile.TileContext(nc) as tc:
        with (
            tc.tile_pool(name="const", bufs=1) as const,
            tc.tile_pool(name="xp", bufs=2) as xp,
            tc.tile_pool(name="thp", bufs=2) as thp,
            tc.tile_pool(name="php", bufs=2) as php,
            tc.tile_pool(name="gp", bufs=2) as gp,
            tc.tile_pool(name="gap", bufs=2) as gap,
            tc.tile_pool(name="ep", bufs=4) as ep,
            tc.tile_pool(name="osp", bufs=2) as osp,
            tc.tile_pool(name="onp", bufs=3) as onp,
            tc.tile_pool(name="o3p", bufs=3) as o3p,
            tc.tile_pool(name="rbp", bufs=3) as rbp,
            tc.tile_pool(name="nrm", bufs=2) as nrm,
            tc.tile_pool(name="drp", bufs=2, space="DRAM") as drp,
            tc.tile_pool(name="pa", bufs=3, space="PSUM") as pa,
            tc.tile_pool(name="pb", bufs=2, space="PSUM") as pb,
        ):
            wt_sb = const.tile([C, C], F16)
            nc.sync.dma_start(wt_sb[:], wtd[:])
            wt2_sb = const.tile([C, C], F16)
            nc.sync.dma_start(wt2_sb[:], wt2d[:])
            wo_sb = const.tile([CG, C], F16)
            nc.sync.dma_start(wo_sb[:], wod[:])
            ident = const.tile([CG, CG], F16)
            nc.sync.dma_start(ident[:], identd[:])
            kbias = const.tile([C, 1], F32)
            nc.vector.memset(kbias[:], -K_SHIFT)

            # ---- phase 1 per sample: projection + pools + g^T ----
            # wt rows: 0:16 theta, 32:48 phi, 64:128 g (32-aligned PSUM).
            stage = {}

            def p1_alloc(b):
                x16 = xp.tile([C, N], F16, name="x16sb")
                for j in range(NJ):
                    nc.sync.dma_start(
                        x16[:, j * NC:(j + 1) * NC],
                        x16d[b][:, j * NC:(j + 1) * NC],
                    )
                # wt rows: 0:16 theta, 16:32 phi, 32:48 theta replica
                # (for the row-group-32 scores tile), 64:128 g.
                th = thp.tile([48, N], F16, name="th")
                ph = php.tile([48, KM // 2, 128], F16, name="ph")
                g = gp.tile([CG, M], F16, name="g")
                ga = gap.tile([C, KM, 2 * CG], F16, name="ga")
                nc.sync.dma_start(ga[:, :, CG:], onec[:])
                stage[b] = (x16, th, ph, g, ga)

            def p1_chunk(b, j):
                x16, th, ph, g, ga = stage[b]
                pp = pa.tile([C, 2, NC], F32, tag="ps", name="pp")
                nc.tensor.matmul(
                    pp[:, 0, :], lhsT=wt_sb[:],
                    rhs=x16[:, j * NC:(j + 1) * NC],
                    start=True, stop=True,
                )
                nc.tensor.matmul(
                    pp[:, 1, :], lhsT=wt2_sb[:],
                    rhs=x16[:, j * NC:(j + 1) * NC],
                    start=True, stop=True,
                )
                # theta copy on ACT (idle during projection work)
                nc.scalar.activation(
                    th[:, j * NC:(j + 1) * NC], pp[0:48, 0, :], AF.Copy
                )
                # phi maxpool: m-tile j -> partition group 32*(j%2)
                vp = pp[0:CT, 1, :].rearrange(
                    "p (r a w b) -> p r w a b", r=4, a=2, b=2
                )
                base = 32 * (j % 2)
                dp = ph[base:base + CT, j // 2, :].rearrange(
                    "p (r w) -> p r w", r=4
                )
                nc.vector.tensor_reduce(
                    dp, vp, axis=mybir.AxisListType.XY, op=ALU.max
                )
                # g maxpool
                vg = pp[64:128, 0, :].rearrange(
                    "p (r a w b) -> p r w a b", r=4, a=2, b=2
                )
                dg = g[:, j * PR:(j + 1) * PR].rearrange(
                    "p (r w) -> p r w", r=4
                )
                nc.vector.tensor_reduce(
                    dg, vg, axis=mybir.AxisListType.XY, op=ALU.max
                )

            def p1_trans(b, k):
                x16, th, ph, g, ga = stage[b]
                pt = pa.tile([C, 2, NC], F16, tag="ps", name="pt")
                nc.tensor.transpose(
                    pt[:, 0, 0:CG], g[:, k * 128:(k + 1) * 128], ident[:]
                )
                nc.scalar.activation(ga[:, k, 0:CG], pt[:, 0, 0:CG], AF.Copy)

            # ---- phase 2 per sample: attention + wo + residual ----
            def phase2(b, filler=None):
                x16, th, ph, g_, ga = stage.pop(b)
                os_t = osp.tile([CG + 1, NJ, NC], F16, name="os_t")
                s_all = nrm.tile([1, N], F32, tag="s_all", name="s_all")
                rinv = nrm.tile([1, N], F32, tag="rinv", name="rinv")
                rscr = drp.tile([1, N], F32, name="rscr")

                for blk in range(NJ // 2):
                    j0, j1 = 2 * blk, 2 * blk + 1
                    po = {
                        j0: pb.tile([C, NC], F32, tag="po", name="po0"),
                        j1: pb.tile([C, NC], F32, tag="po", name="po1"),
                    }
                    prev = None

                    def omm(q, ej0, ej1):
                        # ga k-slice stationary across the two chunks
                        for kk, slot in ((2 * q, 0), (2 * q + 1, 1)):
                            for j, e in ((j0, ej0), (j1, ej1)):
                                nc.tensor.matmul(
                                    po[j][:],
                                    lhsT=ga[:, kk, :],
                                    rhs=e[:, slot, :],
                                    start=(kk == 0),
                                    stop=(kk == KM - 1),
                                )

                    for q in range(KM // 2):
                        ps0 = pa.tile([128, 2, NC], F32, tag="ps", name="ps0")
                        ps1 = pa.tile([128, 2, NC], F32, tag="ps", name="ps1")
                        # k-pair on disjoint PE row groups (0 and 32):
                        # the two matmuls of a pair run concurrently and
                        # the second LDWEIGHTS hides under the first MM.
                        for j, ps in ((j0, ps0), (j1, ps1)):
                            js = slice(j * NC, (j + 1) * NC)
                            nc.tensor.matmul(
                                ps[:, 0, :],
                                lhsT=ph[0:CT, q, :],
                                rhs=th[0:CT, js],
                                start=True, stop=True,
                                tile_position=(0, 0),
                            )
                            nc.tensor.matmul(
                                ps[:, 1, :],
                                lhsT=ph[32:32 + CT, q, :],
                                rhs=th[32:32 + CT, js],
                                start=True, stop=True,
                                tile_position=(32, 0),
                            )
                        e_j0 = ep.tile([128, 2, NC], F16, tag="e", name="e0")
                        e_j1 = ep.tile([128, 2, NC], F16, tag="e", name="e1")
                        nc.scalar.activation(
                            e_j0[:], ps0[:], AF.Exp, bias=kbias[:]
                        )
                        nc.scalar.activation(
                            e_j1[:], ps1[:], AF.Exp, bias=kbias[:]
                        )
                        if prev is not None:
                            omm(*prev)
                        prev = (q, e_j0, e_j1)
                    omm(*prev)

                    for j in (j0, j1):
                        nc.vector.tensor_copy(
                            os_t[:, j, :], po[j][0:CG + 1, :]
                        )
                        nc.vector.tensor_copy(
                            s_all[:, j * NC:(j + 1) * NC],
                            po[j][CG:CG + 1, :],
                        )

                    # per-block softmax denominators -> reciprocal ->
                    # broadcast (DRAM round-trip: DMA replicates 1/den
                    # across partitions).  Normalization commutes through
                    # w_o and is applied after it.
                    bs = slice(j0 * NC, (j1 + 1) * NC)
                    nc.vector.reciprocal_approx_fast(
                        rinv[:, bs], s_all[:, bs]
                    )
                    nc.sync.dma_start(rscr[0:1, bs], rinv[:, bs])
                    for j in (j0, j1):
                        js = slice(j * NC, (j + 1) * NC)
                        rb = rbp.tile([128, NC], F32, name="rb")
                        nc.sync.dma_start(
                            rb[:], rscr[0:1, js].to_broadcast([128, NC])
                        )
                        pf = pa.tile([C, 2, NC], F32, tag="ps", name="pf")
                        nc.tensor.matmul(
                            pf[:, 0, :], lhsT=wo_sb[:], rhs=os_t[0:CG, j, :],
                            start=True, stop=True,
                        )
                        o2 = onp.tile([C, NC], F32, name="o2")
                        nc.vector.tensor_tensor(
                            o2[:], pf[:, 0, :], rb[:], ALU.mult
                        )
                        o3 = o3p.tile([C, NC], F32, name="o3")
                        nc.vector.tensor_tensor(
                            o3[:], o2[:], x16[:, js], ALU.add
                        )
                        nc.sync.dma_start(out[b][:, js], o3[:])
                    if filler is not None:
                        filler(blk)

            for b in range(ns):
                p1_alloc(b)
                for j in range(NJ):
                    p1_chunk(b, j)
                for k in range(KM):
                    p1_trans(b, k)
            for b in range(ns):
                phase2(b)
    nc.finalize()
    return nc


def _prep_inputs(x, w_theta, w_phi, w_g, w_o, gamma):
    x16 = np.ascontiguousarray(
        np.asarray(x, np.float32).reshape(B, C, N).astype(np.float16)
    )
    wt_full = np.zeros((C, C), np.float32)  # padded: 32-aligned PSUM rows
    wt_full[0:CT] = np.asarray(w_theta, np.float32)
    wt_full[32:32 + CT] = np.asarray(w_theta, np.float32)  # row-group-32 copy
    wt_full[64:64 + CG] = np.asarray(w_g, np.float32)
    wt16 = np.ascontiguousarray(wt_full.T.astype(np.float16))
    wt2_full = np.zeros((C, C), np.float32)
    wt2_full[0:CT] = np.asarray(w_phi, np.float32)
    wt2 = np.ascontiguousarray(wt2_full.T.astype(np.float16))
    wo16 = np.ascontiguousarray(
        (np.float32(np.asarray(gamma).reshape(-1)[0])
         * np.asarray(w_o, np.float32)).T.astype(np.float16)
    )  # [64, 128]
    return x16, wt16, wt2, wo16


def _run(x, w_theta, w_phi, w_g, w_o, gamma, trace=False):
    from concourse.bass_utils import run_bass_kernel_spmd

    x16, wt16, wt2, wo16 = _prep_inputs(x, w_theta, w_phi, w_g, w_o, gamma)
    nc = build_nc(NS)
    onec = np.zeros((C, KM, CG), np.float16)
    onec[:, :, 0] = 1.0
    ident = np.eye(CG, dtype=np.float16)
    in_maps = [
        {"x16": np.ascontiguousarray(x16[i * NS:(i + 1) * NS]),
         "wt16": wt16, "wt2": wt2, "wo16": wo16, "onec": onec,
         "ident": ident}
        for i in range(NCORES)
    ]
    res = run_bass_kernel_spmd(nc, in_maps, list(range(NCORES)), trace=trace)
    out = np.concatenate([res.results[i]["out"] for i in range(NCORES)], axis=0)
    return out.reshape(B, C, H, W), res


def kernel(x, w_theta, w_phi, w_g, w_o, gamma):
    out, _ = _run(x, w_theta, w_phi, w_g, w_o, gamma, trace=False)
    return out


# revision 33
# speedup vs baseline: 1.2505x; 1.2505x over previous
"""Trainium2 Bass kernel for nn_Attention_51823075393746.

Self-attention block (SAGAN-style) over x:[16,128,64,64]:
  theta = w_theta @ x            [B, 16, 4096]
  phi   = pool2x2(w_phi @ x)     [B, 16, 1024]
  g     = pool2x2(w_g @ x)       [B, 64, 1024]
  beta  = softmax(theta^T phi)   [B, 4096, 1024]
  out   = gamma * (w_o @ (g @ beta^T)) + x

Sharding: data-parallel over batch, 2 samples per core on 8 cores.

Redesign driven by HW traces (baseline 206us -> ~150us):
  - ACT exp of the 4M-element attention matrix is the hard floor
    (64x [128,2,512] ACTIVATEs ~1.1us each), so the pipeline keeps ACT
    fed and every other engine's work below the PE's.
  - scores matmuls (K=16) are 2-way row-tiled via tile_position: phi
    k-even tiles live at partitions 0:16, k-odd at 32:48, theta is
    replicated to both groups by duplicating w_theta rows inside the
    projection weight; the two matmuls of a pair run CONCURRENTLY on
    disjoint 32-row PE groups (measured: starts 4ns apart).
  - attention runs in j-blocks of 2 chunks; phi/ga k-slices serve the
    two chunks back-to-back, and the o-matmuls for pair q-1 are emitted
    after the scores of pair q, so the PE always has runnable work while
    ACT drains the exp backlog (no head-of-queue blocking).  A dense PE
    stream also keeps the HAM clock gate at 2.4GHz.
  - PSUM (8 banks): scores 2x[128,2,512] + po(j0,j1) 2x[128,512] +
    proj/transpose/wo shared 2x[128,512].  theta/ga PSUM->SBUF copies
    run on ACT (idle during projection); pools on DVE.  NOTE: engine
    PSUM reads must start at a 32-aligned partition (compile-enforced),
    hence the padded projection-weight row layout.
  - softmax: compile-time shift exp(s-6); denominator = ones column in
    ga -> row 64 of po, kept fp32 through a per-block reciprocal +
    DRAM-roundtrip DMA broadcast (GPSIMD partition_broadcast and a
    reciprocal reading from partition base 64 give WRONG RESULTS on HW
    despite passing CoreSim); normalization commutes through w_o.
  - residual from the fp16 x copy on-chip: out = w_o @ o * (1/den) +
    x16, one plain store, no fp32 x load, no DMA read-modify-write.
"""

import sys

for _p in ("/opt/trn_rl_repo",):
    if _p not in sys.path:
        sys.path.insert(0, _p)

import numpy as np

import concourse.bass as bass
import concourse.bacc as bacc
import concourse.mybir as mybir
import concourse.tile as tile

F32 = mybir.dt.float32
F16 = mybir.dt.float16
AF = mybir.ActivationFunctionType
ALU = mybir.AluOpType

B, C, H, W = 16, 128, 64, 64
N = H * W          # 4096 spatial positions
M = N // 4         # 1024 pooled positions
CT = 16            # theta/phi channels (C//8)
CG = 64            # g channels (C//2)
NCORES = 8
NS = B // NCORES   # samples per core
NC = 512           # spatial chunk (free dim of matmuls)
NJ = N // NC       # 8 chunks
KM = M // 128      # 8 m-tiles of pooled positions
PR = NC // 4       # pooled positions produced per chunk (128)
K_SHIFT = 6.0      # constant softmax shift: exp(score - K) keeps fp16 happy


def build_nc(ns: int = NS) -> bass.Bass:
    nc = bacc.Bacc()
    x16d = nc.dram_tensor("x16", [ns, C, N], F16, kind="ExternalInput")
    wtd = nc.dram_tensor("wt16", [C, C], F16, kind="ExternalInput")
    wt2d = nc.dram_tensor("wt2", [C, C], F16, kind="ExternalInput")
    wod = nc.dram_tensor("wo16", [CG, C], F16, kind="ExternalInput")
    onec = nc.dram_tensor("onec", [C, KM, CG], F16, kind="ExternalInput")
    identd = nc.dram_tensor("ident", [CG, CG], F16, kind="ExternalInput")
    out = nc.dram_tensor("out", [ns, C, N], F32, kind="ExternalOutput")

    with tile.TileContext(nc) as tc:
        with (
            tc.tile_pool(name="const", bufs=1) as const,
            tc.tile_pool(name="xp", bufs=2) as xp,
            tc.tile_pool(name="thp", bufs=2) as thp,
            tc.tile_pool(name="php", bufs=2) as php,
            tc.tile_pool(name="gp", bufs=2) as gp,
            tc.tile_pool(name="gap", bufs=2) as gap,
            tc.tile_pool(name="ep", bufs=4) as ep,
            tc.tile_pool(name="osp", bufs=2) as osp,
            tc.tile_pool(name="onp", bufs=4) as onp,
            tc.tile_pool(name="o3p", bufs=4) as o3p,
            tc.tile_pool(name="rbp", bufs=5) as rbp,
            tc.tile_pool(name="nrm", bufs=2) as nrm,
            tc.tile_pool(name="drp", bufs=2, space="DRAM") as drp,
            tc.tile_pool(name="pc", bufs=2, space="PSUM") as pc,
            tc.tile_pool(name="pa", bufs=2, space="PSUM") as pa,
            tc.tile_pool(name="pb", bufs=2, space="PSUM") as pb,
        ):
            wt_sb = const.tile([C, C], F16)
            nc.sync.dma_start(wt_sb[:], wtd[:])
            wt2_sb = const.tile([C, C], F16)
            nc.sync.dma_start(wt2_sb[:], wt2d[:])
            wo_sb = const.tile([CG, C], F16)
            nc.sync.dma_start(wo_sb[:], wod[:])
            ident = const.tile([CG, CG], F16)
            nc.sync.dma_start(ident[:], identd[:])
            kbias = const.tile([C, 1], F32)
            nc.vector.memset(kbias[:], -K_SHIFT)

            # ---- phase 1 per sample: projection + pools + g^T ----
            # wt rows: 0:16 theta, 32:48 phi, 64:128 g (32-aligned PSUM).
            stage = {}

            def p1_alloc(b):
                x16 = xp.tile([C, N], F16, name="x16sb")
                for j in range(NJ):
                    nc.sync.dma_start(
                        x16[:, j * NC:(j + 1) * NC],
                        x16d[b][:, j * NC:(j + 1) * NC],
                    )
                # wt rows: 0:16 theta, 16:32 phi, 32:48 theta replica
                # (for the row-group-32 scores tile), 64:128 g.
                th = thp.tile([48, N], F16, name="th")
                ph = php.tile([48, KM // 2, 128], F16, name="ph")
                g = gp.tile([CG, M], F16, name="g")
                ga = gap.tile([C, KM, 2 * CG], F16, name="ga")
                nc.sync.dma_start(ga[:, :, CG:], onec[:])
                stage[b] = (x16, th, ph, g, ga)

            def p1_chunk(b, j):
                x16, th, ph, g, ga = stage[b]
                pp = pc.tile([C, NC], F32, tag="pc", name="pp")
                nc.tensor.matmul(
                    pp[:], lhsT=wt_sb[:],
                    rhs=x16[:, j * NC:(j + 1) * NC],
                    start=True, stop=True,
                )
                pp2 = pc.tile([C, NC], F32, tag="pc", name="pp2")
                nc.tensor.matmul(
                    pp2[:], lhsT=wt2_sb[:],
                    rhs=x16[:, j * NC:(j + 1) * NC],
                    start=True, stop=True,
                )
                # theta copy on ACT (idle during projection work)
                nc.scalar.activation(
                    th[:, j * NC:(j + 1) * NC], pp[0:48, :], AF.Copy
                )
                # phi maxpool: m-tile j -> partition group 32*(j%2)
                vp = pp2[0:CT].rearrange(
                    "p (r a w b) -> p r w a b", r=4, a=2, b=2
                )
                base = 32 * (j % 2)
                dp = ph[base:base + CT, j // 2, :].rearrange(
                    "p (r w) -> p r w", r=4
                )
                nc.vector.tensor_reduce(
                    dp, vp, axis=mybir.AxisListType.XY, op=ALU.max
                )
                # g maxpool
                vg = pp[64:128].rearrange(
                    "p (r a w b) -> p r w a b", r=4, a=2, b=2
                )
                dg = g[:, j * PR:(j + 1) * PR].rearrange(
                    "p (r w) -> p r w", r=4
                )
                nc.vector.tensor_reduce(
                    dg, vg, axis=mybir.AxisListType.XY, op=ALU.max
                )

            def p1_trans(b, k):
                x16, th, ph, g, ga = stage[b]
                pt = pc.tile([C, NC], F16, tag="pc", name="pt")
                nc.tensor.transpose(
                    pt[:, 0:CG], g[:, k * 128:(k + 1) * 128], ident[:]
                )
                nc.scalar.activation(ga[:, k, 0:CG], pt[:, 0:CG], AF.Copy)

            # ---- phase 2 per sample: attention + wo + residual ----
            def phase2(b, filler=None):
                x16, th, ph, g_, ga = stage.pop(b)
                pending = [None]
                os_t = osp.tile([CG + 1, NJ, NC], F16, name="os_t")
                s_all = nrm.tile([1, N], F32, tag="s_all", name="s_all")
                rinv = nrm.tile([1, N], F32, tag="rinv", name="rinv")
                rscr = drp.tile([1, N], F32, name="rscr")

                for blk in range(NJ // 2):
                    j0, j1 = 2 * blk, 2 * blk + 1
                    po = {
                        j0: pb.tile([C, NC], F32, tag="po", name="po0"),
                        j1: pb.tile([C, NC], F32, tag="po", name="po1"),
                    }
                    prev = None

                    def omm(q, ej0, ej1):
                        # ga k-slice stationary across the two chunks
                        for kk, slot in ((2 * q, 0), (2 * q + 1, 1)):
                            for j, e in ((j0, ej0), (j1, ej1)):
                                nc.tensor.matmul(
                                    po[j][:],
                                    lhsT=ga[:, kk, :],
                                    rhs=e[:, slot, :],
                                    start=(kk == 0),
                                    stop=(kk == KM - 1),
                                )

                    for q in range(KM // 2):
                        ps0 = pa.tile([128, 2, NC], F32, tag="ps", name="ps0")
                        ps1 = pa.tile([128, 2, NC], F32, tag="ps", name="ps1")
                        # k-pair on disjoint PE row groups (0 and 32):
                        # the two matmuls of a pair run concurrently and
                        # the second LDWEIGHTS hides under the first MM.
                        for j, ps in ((j0, ps0), (j1, ps1)):
                            js = slice(j * NC, (j + 1) * NC)
                            nc.tensor.matmul(
                                ps[:, 0, :],
                                lhsT=ph[0:CT, q, :],
                                rhs=th[0:CT, js],
                                start=True, stop=True,
                                tile_position=(0, 0),
                            )
                            nc.tensor.matmul(
                                ps[:, 1, :],
                                lhsT=ph[32:32 + CT, q, :],
                                rhs=th[32:32 + CT, js],
                                start=True, stop=True,
                                tile_position=(32, 0),
                            )
                        e_j0 = ep.tile([128, 2, NC], F16, tag="e", name="e0")
                        e_j1 = ep.tile([128, 2, NC], F16, tag="e", name="e1")
                        nc.scalar.activation(
                            e_j0[:], ps0[:], AF.Exp, bias=kbias[:]
                        )
                        nc.scalar.activation(
                            e_j1[:], ps1[:], AF.Exp, bias=kbias[:]
                        )
                        if prev is not None:
                            omm(*prev)
                        prev = (q, e_j0, e_j1)
                    omm(*prev)

                    for j in (j0, j1):
                        nc.vector.tensor_copy(
                            os_t[:, j, :], po[j][0:CG + 1, :]
                        )
                        nc.vector.tensor_copy(
                            s_all[:, j * NC:(j + 1) * NC],
                            po[j][CG:CG + 1, :],
                        )

                    # per-block softmax denominators -> reciprocal ->
                    # broadcast (DRAM round-trip: DMA replicates 1/den
                    # across partitions).  Normalization commutes through
                    # w_o and is applied after it.
                    bs = slice(j0 * NC, (j1 + 1) * NC)
                    nc.vector.reciprocal_approx_fast(
                        rinv[:, bs], s_all[:, bs]
                    )
                    nc.sync.dma_start(rscr[0:1, bs], rinv[:, bs])
                    rbs = {}
                    for j in (j0, j1):
                        js = slice(j * NC, (j + 1) * NC)
                        rb = rbp.tile([128, NC], F32, name="rb")
                        nc.sync.dma_start(
                            rb[:], rscr[0:1, js].to_broadcast([128, NC])
                        )
                        rbs[j] = rb

                    # the w_o + normalize PE/DVE work is deferred by one
                    # block so its wait on the 1/den DMA round-trip never
                    # head-of-line-blocks the next block's matmuls
                    def tail(j0=j0, j1=j1, rbs=rbs):
                        for j in (j0, j1):
                            js = slice(j * NC, (j + 1) * NC)
                            pf = pc.tile([C, NC], F32, tag="pc", name="pf")
                            nc.tensor.matmul(
                                pf[:], lhsT=wo_sb[:], rhs=os_t[0:CG, j, :],
                                start=True, stop=True,
                            )
                            o2 = onp.tile([C, NC], F32, name="o2")
                            nc.vector.tensor_tensor(
                                o2[:], pf[:], rbs[j][:], ALU.mult
                            )
                            o3 = o3p.tile([C, NC], F32, name="o3")
                            nc.vector.tensor_tensor(
                                o3[:], o2[:], x16[:, js], ALU.add
                            )
                            nc.sync.dma_start(out[b][:, js], o3[:])

                    if pending[0] is not None:
                        pending[0]()
                    pending[0] = tail
                    if filler is not None:
                        filler(blk)
                pending[0]()

            for b in range(ns):
                p1_alloc(b)
                for j in range(NJ):
                    p1_chunk(b, j)
                for k in range(KM):
                    p1_trans(b, k)
            for b in range(ns):
                phase2(b)
    nc.finalize()
    return nc


def _prep_inputs(x, w_theta, w_phi, w_g, w_o, gamma):
    x16 = np.ascontiguousarray(
        np.asarray(x, np.float32).reshape(B, C, N).astype(np.float16)
    )
    wt_full = np.zeros((C, C), np.float32)  # padded: 32-aligned PSUM rows
    wt_full[0:CT] = np.asarray(w_theta, np.float32)
    wt_full[32:32 + CT] = np.asarray(w_theta, np.float32)  # row-group-32 copy
    wt_full[64:64 + CG] = np.asarray(w_g, np.float32)
    wt16 = np.ascontiguousarray(wt_full.T.astype(np.float16))
    wt2_full = np.zeros((C, C), np.float32)
    wt2_full[0:CT] = np.asarray(w_phi, np.float32)
    wt2 = np.ascontiguousarray(wt2_full.T.astype(np.float16))
    wo16 = np.ascontiguousarray(
        (np.float32(np.asarray(gamma).reshape(-1)[0])
         * np.asarray(w_o, np.float32)).T.astype(np.float16)
    )  # [64, 128]
    return x16, wt16, wt2, wo16


def _run(x, w_theta, w_phi, w_g, w_o, gamma, trace=False):
    from concourse.bass_utils import run_bass_kernel_spmd

    x16, wt16, wt2, wo16 = _prep_inputs(x, w_theta, w_phi, w_g, w_o, gamma)
    nc = build_nc(NS)
    onec = np.zeros((C, KM, CG), np.float16)
    onec[:, :, 0] = 1.0
    ident = np.eye(CG, dtype=np.float16)
    in_maps = [
        {"x16": np.ascontiguousarray(x16[i * NS:(i + 1) * NS]),
         "wt16": wt16, "wt2": wt2, "wo16": wo16, "onec": onec,
         "ident": ident}
        for i in range(NCORES)
    ]
    res = run_bass_kernel_spmd(nc, in_maps, list(range(NCORES)), trace=trace)
    out = np.concatenate([res.results[i]["out"] for i in range(NCORES)], axis=0)
    return out.reshape(B, C, H, W), res


def kernel(x, w_theta, w_phi, w_g, w_o, gamma):
    out, _ = _run(x, w_theta, w_phi, w_g, w_o, gamma, trace=False)
    return out
